# revision 34
# baseline (speedup 1.0000x reference)
"""DiT graph-attention block on 8 trn2 NeuronCores.

Sharding: nodes rotated per core so each core's 5120 "local" nodes are rows
0:5120 of its (rotated) input; edges partitioned by dst owner, sorted by dst
chunk (128-node windows) and by src within each chunk (gather locality);
segment softmax/scatter via indicator matmuls with host-precomputed one-hot
tables; src-side k/v/u fetched by dma_gather from a replicated full-node
table; dst-side q/u broadcast per edge from SBUF via indicator matmuls.
Edge MLP + softmax batched per chunk; silu/gelu computed via tanh so the
whole edge phase stays on one activation table set. LN2+MLP deferred to a
batched phase over 512-node groups.
"""
import numpy as np

N, E, D, HEADS, HD, REL, ED, MLPH = 40000, 480000, 128, 8, 16, 64, 32, 512
NC_ = 8
NPAD = 40960          # padded node count (8 * 5120)
NLOC = NPAD // NC_    # 5120 local (padded) nodes per core
NCHUNK = NLOC // 128  # 40 chunks of 128 local nodes
FMC = NPAD // 512     # 80 feature-major chunks in node phase
LOCFM = NLOC // 512   # 10 local fm chunks
HALF = 32768          # int16 index limit for dma_gather

GELU_A = 0.7978845608028654
GELU_B = GELU_A * 0.044715

_f32 = None
_bf16 = None


def _pack_idx16(idx_flat):
    """dma_gather int16 index layout: i -> [i%16, i//16], replicated x8."""
    n = len(idx_flat)
    a = np.zeros((16, n // 16), np.int16)
    a[np.arange(n) % 16, np.arange(n) // 16] = idx_flat
    return np.tile(a, (8, 1))


def _host_pack(edge_index):
    """Per-core edge packing. Returns per-core aux arrays + tile counts."""
    import ml_dtypes
    src_g, dst_g = edge_index[0].astype(np.int64), edge_index[1].astype(np.int64)
    per_core = []
    for ci in range(NC_):
        base = ci * NLOC
        lo_n, hi_n = ci * NLOC, (ci + 1) * NLOC
        m = (dst_g >= lo_n) & (dst_g < hi_n) & (dst_g < N)
        s = src_g[m]  # global ids (kvu table is allgathered in rank order)
        d = dst_g[m] - base  # local 0..NLOC-1
        order = np.argsort(d, kind="stable")
        s, d = s[order], d[order]
        bounds = np.searchsorted(d, np.arange(0, NLOC + 1, 128))
        chunks = []
        for ch in range(NCHUNK):
            a, b = bounds[ch], bounds[ch + 1]
            sl, dl = s[a:b], d[a:b]
            so = np.argsort(sl, kind="stable")  # src-sorted for DRAM locality
            sl, dl = sl[so], dl[so]
            lo = sl < HALF
            chunks.append(((sl[lo], dl[lo]), (sl[~lo], dl[~lo])))
        per_core.append(chunks)
    tlo = max(max((len(c[0][0]) + 127) // 128 for c in chunks)
              for chunks in per_core)
    thi = max(max(max((len(c[1][0]) + 127) // 128, 1) for c in chunks)
              for chunks in per_core)
    TT = tlo + thi
    aux = []
    slot_ar = np.arange(128)
    for ci in range(NC_):
        slo = np.zeros((NCHUNK, tlo * 128), np.int64)
        shi = np.zeros((NCHUNK, thi * 128), np.int64)
        dw = np.full((NCHUNK, TT * 128), -1, np.int64)
        for ch in range(NCHUNK):
            (sl, dl), (sh, dh) = per_core[ci][ch]
            slo[ch, :len(sl)] = sl
            shi[ch, :len(sh)] = sh - HALF
            dw[ch, :len(sl)] = dl - ch * 128
            dw[ch, tlo * 128:tlo * 128 + len(sh)] = dh - ch * 128
        slo16 = np.concatenate([_pack_idx16(slo[ch].astype(np.int16))
                                for ch in range(NCHUNK)], axis=1)
        shi16 = np.concatenate([_pack_idx16(shi[ch].astype(np.int16))
                                for ch in range(NCHUNK)], axis=1)
        didx = np.where(dw >= 0, dw + (np.arange(NCHUNK) * 128)[:, None], 0)
        sd16 = np.concatenate([_pack_idx16(didx[ch].astype(np.int16))
                               for ch in range(NCHUNK)], axis=1)
        # one-hot indicator tables
        dwt = dw.reshape(NCHUNK, TT, 128)  # [ch, t, e] -> slot or -1
        oh = dwt[:, :, :, None] == slot_ar  # [ch, t, e, slot]
        ind = np.ascontiguousarray(
            oh.transpose(2, 0, 1, 3)).reshape(128, NCHUNK * TT * 128)
        indT = np.ascontiguousarray(
            oh.transpose(3, 0, 1, 2)).reshape(128, NCHUNK * TT * 128)
        aux.append(dict(slo16=slo16, shi16=shi16, sd16=sd16,
                        ind=ind.astype(ml_dtypes.bfloat16),
                        indT=indT.astype(ml_dtypes.bfloat16)))
    return tlo, thi, aux


def _build(TLO, THI):
    import concourse.bass as bass
    import concourse.bacc as bacc
    import concourse.mybir as mybir
    from concourse.tile import TileContext
    global _f32, _bf16
    _f32, _bf16 = mybir.dt.float32, mybir.dt.bfloat16
    AF = mybir.ActivationFunctionType
    OP = mybir.AluOpType
    TT = TLO + THI
    L = TT * 128

    nc = bacc.Bacc("TRN2", target_bir_lowering=False, debug=False,
                   num_devices=NC_)
    din = {}
    def I(name, shape, dt=None):
        din[name] = nc.dram_tensor(name, shape, dt or _f32,
                                   kind="ExternalInput")
        return din[name]

    x_in = I("x", [NLOC, D]); c_in = I("c", [NLOC, D])
    for nm, sh in [("wq", [D, D]), ("wk", [D, D]), ("wv", [D, D]),
                   ("wp", [D, D]), ("wrel", [D, REL]), ("wada", [D, 6 * D]),
                   ("w1e", [2 * ED, 3 * 2 * ED]), ("w2e", [2 * ED, ED]),
                   ("wbg", [ED, 2 * HEADS]), ("wf1", [D, MLPH]),
                   ("wf2", [D, MLPH]), ("ones", [128, 128]),
                   ("identb", [128, 128])]:
        I(nm, sh, _bf16)
    I("identf", [128, 128], _f32)
    I("slo16", [128, NCHUNK * TLO * 8], mybir.dt.int16)
    I("shi16", [128, NCHUNK * THI * 8], mybir.dt.int16)
    I("sd16", [128, NCHUNK * (TLO + THI) * 8], mybir.dt.int16)
    I("ind", [128, NCHUNK * L], _bf16)
    I("indT", [128, NCHUNK * L], _bf16)
    y_out = nc.dram_tensor("y", [NLOC, D], _bf16, kind="ExternalOutput")

    # quarter split of the TT tiles for PSUM-sized edge-MLP passes
    qb = [0]
    for i in range(4):
        qb.append(qb[-1] + (TT + 3 - i) // 4)
    quarters = [(qb[i] * 128, qb[i + 1] * 128) for i in range(4)
                if qb[i + 1] > qb[i]]
    QW = max(c1 - c0 for c0, c1 in quarters)

    with TileContext(nc) as tc:
        with (tc.tile_pool(name="const", bufs=1) as cp,
              tc.tile_pool(name="pers", bufs=1) as pp,
              tc.tile_pool(name="dram", bufs=1, space="DRAM") as dp,
              tc.tile_pool(name="work", bufs=2) as wp_,
              tc.tile_pool(name="work2", bufs=2) as wp2,
              tc.tile_pool(name="work1", bufs=1) as wp1,
              tc.tile_pool(name="kvp", bufs=2) as kvp,
              tc.tile_pool(name="ps", bufs=2, space="PSUM") as ps,
              tc.tile_pool(name="ps2", bufs=4, space="PSUM") as ps2,
              tc.tile_pool(name="ps1", bufs=1, space="PSUM") as ps1):
            # PSUM budget (8 banks): big x2, sm x4, pao+st x1, accpmo x1

            # ---- constants / weights into SBUF
            W = {}
            for nm in ["wq", "wk", "wv", "wp", "wrel", "wada", "w1e", "w2e",
                       "wbg", "wf1", "wf2", "ones", "identb"]:
                t = cp.tile(list(din[nm].shape), _bf16, tag=nm)
                nc.sync.dma_start(out=t[:], in_=din[nm][:, :])
                W[nm] = t
            c_eps = cp.tile([128, 1], _f32)
            nc.gpsimd.memset(c_eps[:], 1e-6)
            c_iD = cp.tile([128, 1], _f32)
            nc.gpsimd.memset(c_iD[:], 1.0 / D)
            c_iR = cp.tile([128, 1], _f32)
            nc.gpsimd.memset(c_iR[:], 1.0 / REL)
            c_ga = cp.tile([128, 1], _f32)
            nc.gpsimd.memset(c_ga[:], GELU_A)

            kvu_t = dp.tile([NPAD, 384], _bf16)
            kvu_loc = dp.tile([NLOC, 384], _bf16)
            qd = dp.tile([NLOC, 128], _bf16)
            scfm_d = dp.tile([128, NLOC], _bf16)     # silu(c) fm, local
            h2_d = dp.tile([128, NLOC], _bf16)       # modulated ln2 out

            # persistent local tables
            xf_t = pp.tile([128, NLOC], _bf16)       # x (then x+g*attn) fm
            qu_em = pp.tile([128, NCHUNK * 192], _bf16)  # q|u edge-major

            # ======== PHASE A-s: silu(c) pass (silu table set only) ====
            for g in range(LOCFM):
                r0 = g * 512
                scfm = wp_.tile([128, 512], _bf16, tag="scfm")
                for j in range(4):
                    rr = r0 + j * 128
                    ce = wp_.tile([128, 128], _f32, tag="ce")
                    nc.sync.dma_start(out=ce[:], in_=c_in[rr:rr + 128, :])
                    sce = wp_.tile([128, 128], _bf16, tag="sce")
                    nc.scalar.activation(sce[:], ce[:], AF.Silu)
                    pt2 = ps2.tile([128, 128], _bf16, tag="sm")
                    nc.tensor.transpose(pt2[:], sce[:], W["identb"][:])
                    nc.vector.tensor_copy(out=scfm[:, j * 128:(j + 1) * 128],
                                          in_=pt2[:])
                nc.sync.dma_start(out=scfm_d[:, r0:r0 + 512], in_=scfm[:])

            # ======== PHASE A: node phase (sqrt table set only) ========
            for g in range(LOCFM):
                local = True
                r0 = g * 512
                scg = wp_.tile([128, 512], _bf16, tag="scg")
                nc.sync.dma_start(out=scg[:], in_=scfm_d[:, r0:r0 + 512])
                xe4 = wp_.tile([128, 4, 128], _f32, tag="xe4")
                s14 = wp_.tile([128, 4], _f32, tag="s14")
                s24 = wp_.tile([128, 4], _f32, tag="s24")
                sq4 = wp_.tile([128, 4, 128], _bf16, tag="sq4")
                for j in range(4):
                    rr = r0 + j * 128
                    nc.sync.dma_start(out=xe4[:, j, :],
                                      in_=x_in[rr:rr + 128, :])
                    nc.vector.tensor_reduce(
                        out=s14[:, j:j + 1], in_=xe4[:, j, :],
                        axis=mybir.AxisListType.X, op=OP.add)
                    nc.vector.scalar_tensor_tensor(
                        out=sq4[:, j, :], in0=xe4[:, j, :], scalar=1.0,
                        in1=xe4[:, j, :], op0=OP.mult, op1=OP.mult,
                        accum_out=s24[:, j:j + 1])
                mean4 = wp_.tile([128, 4], _f32, tag="mean4")
                nc.vector.tensor_scalar_mul(out=mean4[:], in0=s14[:],
                                            scalar1=1.0 / D)
                msq4 = wp_.tile([128, 4], _f32, tag="msq4")
                nc.vector.tensor_mul(out=msq4[:], in0=mean4[:], in1=mean4[:])
                var4 = wp_.tile([128, 4], _f32, tag="var4")
                nc.vector.scalar_tensor_tensor(
                    out=var4[:], in0=s24[:], scalar=1.0 / D, in1=msq4[:],
                    op0=OP.mult, op1=OP.subtract)
                sd4 = wp_.tile([128, 4], _f32, tag="sd4")
                nc.scalar.activation(sd4[:], var4[:], AF.Sqrt, bias=c_eps[:])
                rstd4 = wp_.tile([128, 4], _f32, tag="rstd4")
                nc.vector.reciprocal(out=rstd4[:], in_=sd4[:])
                nmr4 = wp_.tile([128, 4], _f32, tag="nmr4")
                nc.vector.scalar_tensor_tensor(
                    out=nmr4[:], in0=mean4[:], scalar=-1.0, in1=rstd4[:],
                    op0=OP.mult, op1=OP.mult)
                ln_fm = wp_.tile([128, 512], _bf16, tag="lnfm")
                for j in range(4):
                    rr = r0 + j * 128
                    lnem = wp_.tile([128, 128], _bf16, tag="lnem")
                    nc.scalar.activation(lnem[:], xe4[:, j, :], AF.Identity,
                                         scale=rstd4[:, j:j + 1],
                                         bias=nmr4[:, j:j + 1])
                    pt = ps2.tile([128, 128], _bf16, tag="sm")
                    nc.tensor.transpose(pt[:], lnem[:], W["identb"][:])
                    nc.vector.tensor_copy(out=ln_fm[:, j * 128:(j + 1) * 128],
                                          in_=pt[:])
                    if local:
                        ptx = ps2.tile([128, 128], _bf16, tag="sm")
                        xbe = wp_.tile([128, 128], _bf16, tag="xbe")
                        nc.vector.tensor_copy(out=xbe[:], in_=xe4[:, j, :])
                        nc.tensor.transpose(ptx[:], xbe[:], W["identb"][:])
                        nc.vector.tensor_copy(
                            out=xf_t[:, rr:rr + 128], in_=ptx[:])
                # ada slices 0,1 (used as sc, sh)
                pa0 = ps.tile([128, 512], _f32, tag="big")
                nc.tensor.matmul(pa0[:], W["wada"][:, 0:128], scg[:],
                                 start=True, stop=True)
                pa1 = ps.tile([128, 512], _f32, tag="big")
                nc.tensor.matmul(pa1[:], W["wada"][:, 128:256], scg[:],
                                 start=True, stop=True)
                t3 = wp_.tile([128, 512], _bf16, tag="t3")
                nc.vector.scalar_tensor_tensor(
                    out=t3[:], in0=pa0[:], scalar=1.0, in1=ln_fm[:],
                    op0=OP.add, op1=OP.mult)
                h_bf = wp_.tile([128, 512], _bf16, tag="hbf")
                nc.vector.tensor_add(out=h_bf[:], in0=t3[:], in1=pa1[:])
                # k, v
                stage = wp2.tile([128, 4, 384], _bf16, tag="stage")
                for nm, off in [("wk", 0), ("wv", 192)]:
                    pk = ps.tile([128, 512], _f32, tag="big")
                    nc.tensor.matmul(pk[:], W[nm][:], h_bf[:], start=True,
                                     stop=True)
                    ksb = wp_.tile([128, 512], _bf16, tag="ksb")
                    nc.scalar.activation(ksb[:], pk[:], AF.Copy)
                    for j in range(4):
                        ptk = ps2.tile([128, 128], _bf16, tag="sm")
                        nc.tensor.transpose(
                            ptk[:], ksb[:, j * 128:(j + 1) * 128],
                            W["identb"][:])
                        nc.vector.tensor_copy(
                            out=stage[:, j, off:off + 128], in_=ptk[:])
                # u: rel proj + LN(em) + store
                pu = ps.tile([64, 512], _f32, tag="big")
                nc.tensor.matmul(pu[:], W["wrel"][:], h_bf[:], start=True,
                                 stop=True)
                usb = wp_.tile([64, 512], _bf16, tag="usb")
                nc.scalar.activation(usb[:], pu[:], AF.Copy)
                usm = wp_.tile([128, 4, 64], _bf16, tag="usm")
                us14 = wp_.tile([128, 4], _f32, tag="us14")
                us24 = wp_.tile([128, 4], _f32, tag="us24")
                usq4 = wp_.tile([128, 4, 64], _bf16, tag="usq4")
                for j in range(4):
                    put = ps2.tile([128, 64], _bf16, tag="sm")
                    nc.tensor.transpose(put[:], usb[:, j * 128:(j + 1) * 128],
                                        W["identb"][:64, :64])
                    nc.vector.tensor_copy(out=usm[:, j, :], in_=put[:])
                    nc.vector.tensor_reduce(
                        out=us14[:, j:j + 1], in_=usm[:, j, :],
                        axis=mybir.AxisListType.X, op=OP.add)
                    nc.vector.scalar_tensor_tensor(
                        out=usq4[:, j, :], in0=usm[:, j, :], scalar=1.0,
                        in1=usm[:, j, :], op0=OP.mult, op1=OP.mult,
                        accum_out=us24[:, j:j + 1])
                um4 = wp_.tile([128, 4], _f32, tag="um4")
                nc.vector.tensor_scalar_mul(out=um4[:], in0=us14[:],
                                            scalar1=1.0 / REL)
                umq4 = wp_.tile([128, 4], _f32, tag="umq4")
                nc.vector.tensor_mul(out=umq4[:], in0=um4[:], in1=um4[:])
                uva4 = wp_.tile([128, 4], _f32, tag="uva4")
                nc.vector.scalar_tensor_tensor(
                    out=uva4[:], in0=us24[:], scalar=1.0 / REL, in1=umq4[:],
                    op0=OP.mult, op1=OP.subtract)
                usd4 = wp_.tile([128, 4], _f32, tag="usd4")
                nc.scalar.activation(usd4[:], uva4[:], AF.Sqrt, bias=c_eps[:])
                urs4 = wp_.tile([128, 4], _f32, tag="urs4")
                nc.vector.reciprocal(out=urs4[:], in_=usd4[:])
                unm4 = wp_.tile([128, 4], _f32, tag="unm4")
                nc.vector.scalar_tensor_tensor(
                    out=unm4[:], in0=um4[:], scalar=-1.0, in1=urs4[:],
                    op0=OP.mult, op1=OP.mult)
                for j in range(4):
                    nc.vector.tensor_scalar(
                        out=stage[:, j, 128:192], in0=usm[:, j, :],
                        scalar1=urs4[:, j:j + 1], scalar2=unm4[:, j:j + 1],
                        op0=OP.mult, op1=OP.add)
                nc.gpsimd.dma_start(
                    out=kvu_loc[g * 512:(g + 1) * 512, :].rearrange(
                        "(j p) f -> p j f", p=128),
                    in_=stage[:])
                if local:
                    pq = ps.tile([128, 512], _f32, tag="big")
                    nc.tensor.matmul(pq[:], W["wq"][:], h_bf[:], start=True,
                                     stop=True)
                    qsb = wp_.tile([128, 512], _bf16, tag="qsb")
                    nc.scalar.activation(qsb[:], pq[:], AF.Copy)
                    for j in range(4):
                        ch = g * 4 + j
                        ptq = ps2.tile([128, 128], _bf16, tag="sm")
                        nc.tensor.transpose(
                            ptq[:], qsb[:, j * 128:(j + 1) * 128],
                            W["identb"][:])
                        nc.vector.tensor_copy(
                            out=qu_em[:, ch * 192:ch * 192 + 128], in_=ptq[:])
                        nc.vector.tensor_copy(
                            out=qu_em[:, ch * 192 + 128:ch * 192 + 192],
                            in_=stage[:, j, 128:192])

            # ---- q table to DRAM for the dst-side per-edge gather
            for ch in range(NCHUNK):
                nc.sync.dma_start(out=qd[ch * 128:(ch + 1) * 128, :],
                                  in_=qu_em[:, ch * 192:ch * 192 + 128])

            # ---- AllGather local kvu tables into the full node table
            nc.gpsimd.collective_compute(
                kind="AllGather", op=OP.bypass,
                replica_groups=[list(range(NC_))],
                ins=[kvu_loc[:, :]], outs=[kvu_t[:, :]])

            # ======== PHASE B: edge phase (batched per chunk) ========
            scale = float(HD) ** -0.5
            for ch in range(NCHUNK):
                co = ch * 128
                qcol = ch * 192
                ind_sb = wp2.tile([128, L], _bf16, tag="ind")
                nc.sync.dma_start(out=ind_sb[:],
                                  in_=din["ind"][:, ch * L:(ch + 1) * L])
                indT_sb = wp2.tile([128, L], _bf16, tag="indT")
                nc.sync.dma_start(out=indT_sb[:],
                                  in_=din["indT"][:, ch * L:(ch + 1) * L])
                slo_sb = wp2.tile([128, TLO * 8], mybir.dt.int16, tag="slo")
                nc.sync.dma_start(
                    out=slo_sb[:],
                    in_=din["slo16"][:, ch * TLO * 8:(ch + 1) * TLO * 8])
                shi_sb = wp2.tile([128, THI * 8], mybir.dt.int16, tag="shi")
                nc.sync.dma_start(
                    out=shi_sb[:],
                    in_=din["shi16"][:, ch * THI * 8:(ch + 1) * THI * 8])
                sd_sb = wp2.tile([128, TT * 8], mybir.dt.int16, tag="sd")
                nc.sync.dma_start(
                    out=sd_sb[:],
                    in_=din["sd16"][:, ch * TT * 8:(ch + 1) * TT * 8])
                kvg = kvp.tile([128, TT, 384], _bf16, tag="kvg")
                nc.gpsimd.dma_gather(
                    out_ap=kvg[:, 0:TLO, :], in_ap=kvu_t[0:HALF, :],
                    idxs_ap=slo_sb[:],
                    num_idxs=TLO * 128, num_idxs_reg=TLO * 128,
                    elem_size=384, single_packet=False)
                nc.gpsimd.dma_gather(
                    out_ap=kvg[:, TLO:TT, :], in_ap=kvu_t[HALF:NPAD, :],
                    idxs_ap=shi_sb[:],
                    num_idxs=THI * 128, num_idxs_reg=THI * 128,
                    elem_size=384, single_packet=False)
                # u_i / u_j feature-major
                ui = wp2.tile([64, L], _bf16, tag="ui")
                uj = wp2.tile([64, L], _bf16, tag="uj")
                for c0, c1 in quarters:
                    pui = ps.tile([64, QW], _f32, tag="big")
                    nc.tensor.matmul(pui[:, 0:c1 - c0],
                                     qu_em[:, qcol + 128:qcol + 192],
                                     indT_sb[:, c0:c1], start=True, stop=True)
                    nc.scalar.activation(ui[:, c0:c1], pui[:, 0:c1 - c0],
                                         AF.Copy)
                for t in range(TT):
                    pT = ps2.tile([64, 128], _bf16, tag="sm")
                    nc.tensor.transpose(pT[:], kvg[:, t, 128:192],
                                        W["identb"][:])
                    nc.scalar.activation(uj[:, t * 128:(t + 1) * 128],
                                         pT[:], AF.Copy)
                ad = wp2.tile([64, L], _bf16, tag="ad")
                nc.vector.tensor_tensor(out=ad[:], in0=ui[:], in1=uj[:],
                                        op=OP.subtract)
                nc.scalar.activation(ad[:], ad[:], AF.Abs)
                # edge MLP, batched; silu(x) = x*(1+tanh(x/2)) [*0.5 in w2e]
                ef2 = wp2.tile([32, L], _bf16, tag="ef2")
                bgf = wp2.tile([16, L], _bf16, tag="bgf")
                for c0, c1 in quarters:
                    w_q = c1 - c0
                    pe1 = ps.tile([64, QW], _f32, tag="big")
                    for i3, src in enumerate((ui, uj, ad)):
                        nc.tensor.matmul(
                            pe1[:, 0:w_q], W["w1e"][:, i3 * 64:(i3 + 1) * 64],
                            src[:, c0:c1], start=(i3 == 0), stop=(i3 == 2))
                    th1 = wp2.tile([64, QW], _bf16, tag="th1")
                    nc.scalar.activation(th1[:, 0:w_q], pe1[:, 0:w_q],
                                         AF.Tanh, scale=0.5)
                    ef1 = wp2.tile([64, QW], _bf16, tag="ef1")
                    nc.vector.scalar_tensor_tensor(
                        out=ef1[:, 0:w_q], in0=th1[:, 0:w_q], scalar=1.0,
                        in1=pe1[:, 0:w_q], op0=OP.add, op1=OP.mult)
                    pe2 = ps.tile([32, QW], _f32, tag="big")
                    nc.tensor.matmul(pe2[:, 0:w_q], W["w2e"][:],
                                     ef1[:, 0:w_q], start=True, stop=True)
                    nc.scalar.activation(ef2[:, c0:c1], pe2[:, 0:w_q],
                                         AF.Copy)
                    pbg = ps.tile([16, QW], _f32, tag="big")
                    nc.tensor.matmul(pbg[:, 0:w_q], W["wbg"][:],
                                     ef2[:, c0:c1], start=True, stop=True)
                    nc.scalar.activation(bgf[:, c0:c1], pbg[:, 0:w_q],
                                         AF.Copy)
                # bias/gate to edge-major
                bg_em = wp2.tile([128, TT, 16], _bf16, tag="bgem")
                for t in range(TT):
                    pbt = ps2.tile([128, 16], _bf16, tag="sm")
                    nc.tensor.transpose(pbt[:], bgf[:, t * 128:(t + 1) * 128],
                                        W["identb"][:16, :16])
                    nc.vector.tensor_copy(out=bg_em[:, t, :], in_=pbt[:])
                # per-edge q via dst-local gather; batched q*k and reduce
                qeg = wp2.tile([128, TT, 128], _bf16, tag="qeg")
                nc.gpsimd.dma_gather(
                    out_ap=qeg[:], in_ap=qd[:, :], idxs_ap=sd_sb[:],
                    num_idxs=TT * 128, num_idxs_reg=TT * 128,
                    elem_size=128, single_packet=False)
                nc.vector.tensor_mul(out=qeg[:], in0=qeg[:],
                                     in1=kvg[:, :, 0:128])
                sim = wp_.tile([128, TT, 8], _f32, tag="sim")
                nc.vector.tensor_reduce(
                    out=sim[:], in_=qeg[:].rearrange("p t (h d) -> p t h d",
                                                     h=8),
                    axis=mybir.AxisListType.X, op=OP.add)
                sb_ = wp_.tile([128, TT, 8], _f32, tag="sb_")
                nc.vector.scalar_tensor_tensor(
                    out=sb_[:], in0=sim[:], scalar=scale,
                    in1=bg_em[:, :, 0:8], op0=OP.mult, op1=OP.add)
                w_ = wp_.tile([128, TT, 8], _bf16, tag="w_")
                nc.scalar.activation(w_[:], sb_[:], AF.Exp)
                th = wp_.tile([128, TT, 8], _f32, tag="th")
                nc.scalar.activation(th[:], bg_em[:, :, 8:16], AF.Tanh)
                wg = wp_.tile([128, TT, 8], _f32, tag="wg")
                nc.vector.scalar_tensor_tensor(
                    out=wg[:], in0=th[:], scalar=1.0, in1=w_[:],
                    op0=OP.add, op1=OP.mult)
                msgw = wp2.tile([128, TT, 136], _bf16, tag="msgw")
                nc.vector.tensor_mul(
                    out=msgw[:, :, 0:128].rearrange("p t (h d) -> p t h d",
                                                    h=8),
                    in0=kvg[:, :, 192:320].rearrange("p t (h d) -> p t h d",
                                                     h=8),
                    in1=wg[:, :, :, None].to_broadcast([128, TT, 8, 16]))
                nc.vector.tensor_copy(out=msgw[:, :, 128:136], in_=w_[:])
                acc = ps1.tile([128, 136], _f32, tag="accpmo")
                for t in range(TT):
                    nc.tensor.matmul(acc[:], ind_sb[:, t * 128:(t + 1) * 128],
                                     msgw[:, t, :], start=(t == 0),
                                     stop=(t == TT - 1))
                # ---- close chunk: normalize, proj, residual into xf_t
                de = wp_.tile([128, 8], _f32, tag="de")
                nc.vector.tensor_scalar_add(out=de[:], in0=acc[:, 128:136],
                                            scalar1=1e-16)
                r = wp_.tile([128, 8], _f32, tag="r")
                nc.vector.reciprocal(out=r[:], in_=de[:])
                agg = wp_.tile([128, 128], _bf16, tag="agg")
                nc.vector.tensor_mul(
                    out=agg[:].rearrange("p (h d) -> p h d", h=8),
                    in0=acc[:, 0:128].rearrange("p (h d) -> p h d", h=8),
                    in1=r[:, :, None].to_broadcast([128, 8, 16]))
                pag = ps2.tile([128, 128], _bf16, tag="sm")
                nc.tensor.transpose(pag[:], agg[:], W["identb"][:])
                agf = wp_.tile([128, 128], _bf16, tag="agf")
                nc.scalar.activation(agf[:], pag[:], AF.Copy)
                pao = ps1.tile([128, 128], _f32, tag="pao")
                nc.tensor.matmul(pao[:], W["wp"][:], agf[:], start=True,
                                 stop=True)
                ao_sb = wp_.tile([128, 128], _f32, tag="ao_sb")
                nc.vector.tensor_copy(out=ao_sb[:], in_=pao[:])
                scb = wp_.tile([128, 128], _bf16, tag="scb")
                nc.sync.dma_start(out=scb[:], in_=scfm_d[:, co:co + 128])
                pgm = ps2.tile([128, 128], _f32, tag="sm")
                nc.tensor.matmul(pgm[:], W["wada"][:, 256:384],
                                 scb[:], start=True, stop=True)
                t4 = wp_.tile([128, 128], _f32, tag="t4")
                nc.vector.scalar_tensor_tensor(
                    out=t4[:], in0=pgm[:], scalar=1.0, in1=ao_sb[:],
                    op0=OP.mult, op1=OP.mult)
                nc.vector.tensor_add(out=xf_t[:, co:co + 128],
                                     in0=xf_t[:, co:co + 128], in1=t4[:])

            # ======== PHASE C1: LN2 + modulate (sqrt set) ========
            for g in range(LOCFM):
                r0 = g * 512
                xub = wp2.tile([128, 512], _bf16, tag="xub")
                nc.vector.tensor_copy(out=xub[:], in_=xf_t[:, r0:r0 + 512])
                squ = wp2.tile([128, 512], _bf16, tag="squ")
                nc.scalar.activation(squ[:], xub[:], AF.Square)
                pst = ps1.tile([1, 512], _f32, tag="pao")
                nc.tensor.matmul(pst[:], W["ones"][:, 0:1], xub[:],
                                 start=True, stop=True)
                psq = ps1.tile([1, 512], _f32, tag="pao")
                nc.tensor.matmul(psq[:], W["ones"][:, 0:1], squ[:],
                                 start=True, stop=True)
                mn = wp1.tile([1, 512], _bf16, tag="mnS")
                nc.vector.tensor_scalar_mul(out=mn[:], in0=pst[:],
                                            scalar1=1.0 / D)
                mq = wp1.tile([1, 512], _bf16, tag="mqS")
                nc.vector.tensor_mul(out=mq[:], in0=mn[:], in1=mn[:])
                vr = wp1.tile([1, 512], _bf16, tag="vrS")
                nc.vector.scalar_tensor_tensor(
                    out=vr[:], in0=psq[:], scalar=1.0 / D,
                    in1=mq[:], op0=OP.mult, op1=OP.subtract)
                sd2 = wp1.tile([1, 512], _bf16, tag="sdS")
                nc.scalar.activation(sd2[:], vr[:], AF.Sqrt, bias=c_eps[:1])
                rstd = wp1.tile([1, 512], _f32, tag="rsS")
                nc.vector.reciprocal(out=rstd[:], in_=sd2[:])
                rs2 = wp1.tile([1, 512], _bf16, tag="rs2")
                nc.vector.tensor_copy(out=rs2[:], in_=rstd[:])
                nm2 = wp1.tile([1, 512], _bf16, tag="nm2")
                nc.vector.scalar_tensor_tensor(
                    out=nm2[:], in0=mn[:], scalar=-1.0,
                    in1=rstd[:], op0=OP.mult, op1=OP.mult)
                prb = ps.tile([128, 512], _f32, tag="big")
                nc.tensor.matmul(prb[:], W["ones"][0:1, :], rs2[:],
                                 start=True, stop=True)
                pnb = ps.tile([128, 512], _f32, tag="big")
                nc.tensor.matmul(pnb[:], W["ones"][0:1, :], nm2[:],
                                 start=True, stop=True)
                l1 = wp2.tile([128, 512], _bf16, tag="l1")
                nc.vector.tensor_mul(out=l1[:], in0=xub[:], in1=prb[:])
                l2 = wp2.tile([128, 512], _bf16, tag="l2")
                nc.vector.tensor_add(out=l2[:], in0=l1[:], in1=pnb[:])
                scc = wp2.tile([128, 512], _bf16, tag="scc")
                nc.sync.dma_start(out=scc[:], in_=scfm_d[:, r0:r0 + 512])
                psc = ps.tile([128, 512], _f32, tag="big")
                nc.tensor.matmul(psc[:], W["wada"][:, 384:512],
                                 scc[:], start=True, stop=True)
                psh = ps.tile([128, 512], _f32, tag="big")
                nc.tensor.matmul(psh[:], W["wada"][:, 512:640],
                                 scc[:], start=True, stop=True)
                t5 = wp2.tile([128, 512], _bf16, tag="t5")
                nc.vector.scalar_tensor_tensor(
                    out=t5[:], in0=psc[:], scalar=1.0, in1=l2[:],
                    op0=OP.add, op1=OP.mult)
                h2g = wp2.tile([128, 512], _bf16, tag="h2g")
                nc.vector.tensor_add(out=h2g[:], in0=t5[:], in1=psh[:])
                nc.sync.dma_start(out=h2_d[:, r0:r0 + 512], in_=h2g[:])

            # ======== PHASE C2: MLP + residual + output (exp set) ========
            for g in range(LOCFM):
                r0 = g * 512
                h2r = wp2.tile([128, 512], _bf16, tag="h2r")
                nc.sync.dma_start(out=h2r[:], in_=h2_d[:, r0:r0 + 512])
                pmo = ps1.tile([128, 512], _f32, tag="accpmo")
                for jm in range(4):
                    pm1 = ps.tile([128, 512], _f32, tag="big")
                    nc.tensor.matmul(pm1[:],
                                     W["wf1"][:, jm * 128:(jm + 1) * 128],
                                     h2r[:], start=True,
                                     stop=True)
                    # gelu(x) = x*(1+tanh(x*(A + B*x^2))) [*0.5 in wf2]
                    x2 = wp2.tile([128, 512], _bf16, tag="x2")
                    nc.scalar.activation(x2[:], pm1[:], AF.Square)
                    inA = wp2.tile([128, 512], _bf16, tag="inA")
                    nc.scalar.activation(inA[:], x2[:], AF.Identity,
                                         scale=GELU_B, bias=c_ga[:])
                    inB = wp2.tile([128, 512], _bf16, tag="inB")
                    nc.vector.tensor_mul(out=inB[:], in0=inA[:], in1=pm1[:])
                    tg = wp2.tile([128, 512], _bf16, tag="tg")
                    nc.scalar.activation(tg[:], inB[:], AF.Tanh)
                    gl = wp2.tile([128, 512], _bf16, tag="gl")
                    nc.vector.scalar_tensor_tensor(
                        out=gl[:], in0=tg[:], scalar=1.0, in1=pm1[:],
                        op0=OP.add, op1=OP.mult)
                    nc.tensor.matmul(pmo[:],
                                     W["wf2"][:, jm * 128:(jm + 1) * 128],
                                     gl[:], start=(jm == 0), stop=(jm == 3))
                scc2 = wp2.tile([128, 512], _bf16, tag="scc")
                nc.sync.dma_start(out=scc2[:], in_=scfm_d[:, r0:r0 + 512])
                pgl = ps.tile([128, 512], _f32, tag="big")
                nc.tensor.matmul(pgl[:], W["wada"][:, 640:768],
                                 scc2[:], start=True, stop=True)
                glm = wp2.tile([128, 512], _bf16, tag="glm")
                nc.scalar.activation(glm[:], pgl[:], AF.Copy)
                t6 = wp2.tile([128, 512], _bf16, tag="t6")
                nc.vector.tensor_mul(out=t6[:], in0=glm[:], in1=pmo[:])
                yf = wp2.tile([128, 512], _bf16, tag="yf")
                nc.vector.tensor_add(out=yf[:], in0=xf_t[:, r0:r0 + 512],
                                     in1=t6[:])
                ystage = wp2.tile([128, 4, 128], _bf16, tag="ystage")
                for j in range(4):
                    pye = ps2.tile([128, 128], _bf16, tag="sm")
                    nc.tensor.transpose(pye[:],
                                        yf[:, j * 128:(j + 1) * 128],
                                        W["identb"][:])
                    nc.vector.tensor_copy(out=ystage[:, j, :], in_=pye[:])
                nc.gpsimd.dma_start(
                    out=y_out[r0:r0 + 512, :].rearrange("(j p) f -> p j f",
                                                        p=128),
                    in_=ystage[:])
    nc.compile()
    return nc


_CACHE = {}
LAST_RESULT = None  # BassKernelResults of most recent run (for test harness)


def kernel(**inputs):
    global LAST_RESULT
    import concourse.mybir as mybir
    from concourse.bass_utils import run_bass_kernel_spmd

    x = np.asarray(inputs["x"], np.float32)
    c = np.asarray(inputs["c"], np.float32)
    ei = np.asarray(inputs["edge_index"])
    TLO, THI, aux = _host_pack(ei)

    import ml_dtypes
    def b16(a):
        return np.asarray(a, np.float32).astype(ml_dtypes.bfloat16)

    key = (TLO, THI)
    if key not in _CACHE:
        _CACHE[key] = _build(TLO, THI)
    nc = _CACHE[key]

    xp = np.zeros((NPAD, D), np.float32); xp[:N] = x
    cp_ = np.zeros((NPAD, D), np.float32); cp_[:N] = c
    ones = np.ones((128, 128), np.float32)
    ident = np.eye(128, dtype=np.float32)
    wbg = np.concatenate([inputs["Wbias"], inputs["Wgate"]], axis=1)

    common = dict(
        wq=b16(inputs["Wq"]), wk=b16(inputs["Wk"]), wv=b16(inputs["Wv"]),
        wp=b16(inputs["Wp"]), wrel=b16(inputs["Wrel"]),
        wada=b16(inputs["Wada"]), w1e=b16(np.concatenate([inputs["W1e"][0:64],
            inputs["W1e"][64:128], inputs["W1e"][128:192]], axis=1)),
        w2e=b16(0.5 * np.asarray(inputs["W2e"], np.float32)),
        wbg=b16(wbg), wf1=b16(inputs["Wf1"]),
        wf2=b16(np.concatenate([0.5 * np.asarray(inputs["Wf2"], np.float32)
            [i * 128:(i + 1) * 128] for i in range(4)], axis=1)),
        ones=b16(ones), identb=b16(ident), identf=ident)

    in_maps = []
    for ci in range(NC_):
        im = dict(common)
        im["x"] = xp[ci * NLOC:(ci + 1) * NLOC]
        im["c"] = cp_[ci * NLOC:(ci + 1) * NLOC]
        im["slo16"] = aux[ci]["slo16"]
        im["shi16"] = aux[ci]["shi16"]
        im["sd16"] = aux[ci]["sd16"]
        im["ind"] = aux[ci]["ind"]
        im["indT"] = aux[ci]["indT"]
        in_maps.append(im)

    res = run_bass_kernel_spmd(nc, in_maps, core_ids=list(range(NC_)))
    LAST_RESULT = res
    out = np.zeros((N, D), np.float32)
    for ci in range(NC_):
        lo = ci * NLOC
        hi = min(lo + NLOC, N)
        out[lo:hi] = np.asarray(res.results[ci]["y"][:hi - lo], np.float32)
    return out
